# revision 13
# baseline (speedup 1.0000x reference)
"""Trainium2 Bass kernel for nn_NEURAL_PYSCF_WF (neural wavefunction).

reference:
  mo   = einsum('ben,mn->bem', ao, mo_weight)          # [B, 32, 128]
  sub  = mo[:, cfg[:,:,None], cfg[:,None,:]]           # [B, 128, 16, 16]
  dets = det(sub)                                      # [B, 128]
  out  = dets @ ci_weight.T                            # [B, 1]

Config indices are < 32, so only mo[:, :, :32] matters.

Strategy (8 NeuronCores, data-parallel over B=8192). Per core (1024 rows):
  phase 1: ao tiles -> PE transpose -> matmul (W32T stationary) ->
           M^T [m,(b,e)] in fp16 -> DRAM scratch; reload per 128-row
           b-tile as M [128b, 1024(e,m)] fp16.
  phase 2 per b-tile (128 walkers in partitions):
    per c8 sub-chunk: gpsimd ap_gather (d=16, fp16) config rows ->
      R [c,i,m32] fp16; ScalarE strided copy-transpose+cast ->
      Rt [c,m32,i16] fp32; gpsimd ap_gather (d=16, fp32) config cols ->
      sub [c,j,i] fp32 (det(A^T)==det(A)).
    LU: pivot-free elimination over chunks of 32 configs, two chunks
      interleaved so cross-engine gaps are filled.  Reciprocal clamped
      to +-1e6.  The S -= L*row update for the biggest steps
      (k in KS_GPS) runs on GPSIMD in parallel with DVE.
      Trailing 2x2 determinant in closed form; product tree in fp32.
  out[b] = sum_c ci[c] * det[b, c]  (TT mult + reduce).
"""

from contextlib import ExitStack

import numpy as np

import concourse.bass as bass
import concourse.bacc as bacc
import concourse.mybir as mybir
import concourse.tile as tile
from concourse.bass_utils import run_bass_kernel_spmd

F32 = mybir.dt.float32
F16 = mybir.dt.float16
I16 = mybir.dt.int16
AX = mybir.AxisListType
OP = mybir.AluOpType

B = 8192
NE = 32      # electrons (and the max config index)
NAO = 128
K = 16       # config size
NCONF = 128
NCORES = 8
BC = B // NCORES
RCLAMP = 1e6
CG = 8       # configs per gather chunk
CL = 32      # configs per LU chunk
# elimination steps whose S-update runs on GPSIMD, per stream (A, B).
# Asymmetric so the per-step DVE block stays ahead of GPSIMD.
KS_GPS_A = tuple(range(9))
KS_GPS_B = tuple(range(3))


def wrap_idx(idx: np.ndarray) -> np.ndarray:
    """Wrap a flat index list into ap_gather's [128, n/16] layout."""
    n = idx.shape[0]
    assert n % 16 == 0
    w = idx.reshape(n // 16, 16).T.astype(np.int16)
    return np.tile(w, (8, 1))


def build_gidx1(cfg: np.ndarray) -> np.ndarray:
    """Stage-1 indices per c8 chunk: (c,i,h) -> block cfg[c,i]*2+h."""
    cols = []
    for ch in range(NCONF // CG):
        sl = cfg[ch * CG:(ch + 1) * CG]                    # [CG, 16]
        idx = (sl[:, :, None].astype(np.int64) * 2
               + np.arange(2)[None, None, :]).reshape(-1)  # CG*K*2 = 256
        cols.append(wrap_idx(idx))                         # [128, 16]
    return np.concatenate(cols, axis=1)                    # [128, 256]


def build_gidx2(cfg: np.ndarray) -> np.ndarray:
    """Stage-2 indices per c8 chunk: (c_local, j) -> c_local*32 + cfg[c,j]."""
    cols = []
    for ch in range(NCONF // CG):
        sl = cfg[ch * CG:(ch + 1) * CG]                    # [CG, 16]
        idx = (np.arange(CG)[:, None] * NE + sl).reshape(-1)   # CG*K = 128
        cols.append(wrap_idx(idx))                         # [128, 8]
    return np.concatenate(cols, axis=1)                    # [128, 128]


def emit_program(nc, tc, aps, BCc: int):
    ctx = ExitStack()
    NBT = BCc // 128
    NCH = NCONF // CL          # LU chunks per btile (4)
    NG = CL // CG              # gather chunks per LU chunk (4)
    ao, w32t, ident, cirep, gidx1, gidx2, mscr, out = (
        aps["ao"], aps["w32t"], aps["ident"], aps["cirep"], aps["gidx1"],
        aps["gidx2"], aps["mscr"], aps["out"])

    with ctx:
        cpool = ctx.enter_context(tc.tile_pool(name="consts", bufs=1))
        nat = ctx.enter_context(tc.tile_pool(name="nat", bufs=4))
        tp_ps = ctx.enter_context(
            tc.tile_pool(name="tp_ps", bufs=3, space="PSUM"))
        aot = ctx.enter_context(tc.tile_pool(name="aot", bufs=2))
        m_ps = ctx.enter_context(
            tc.tile_pool(name="m_ps", bufs=3, space="PSUM"))
        msb = ctx.enter_context(tc.tile_pool(name="msb", bufs=1))
        rp = ctx.enter_context(tc.tile_pool(name="rp", bufs=1))
        rtp = ctx.enter_context(tc.tile_pool(name="rtp", bufs=1))
        subp = ctx.enter_context(tc.tile_pool(name="subp", bufs=1))
        lb = ctx.enter_context(tc.tile_pool(name="lb", bufs=1))
        pb = ctx.enter_context(tc.tile_pool(name="pb", bufs=1))
        sm = ctx.enter_context(tc.tile_pool(name="sm", bufs=2))
        dets = ctx.enter_context(tc.tile_pool(name="dets", bufs=2))
        outp = ctx.enter_context(tc.tile_pool(name="outp", bufs=1))

        w32t_s = cpool.tile([128, NE], F32)
        ident_s = cpool.tile([128, 128], F32)
        cirep_s = cpool.tile([128, NCONF], F32)
        gidx1_s = cpool.tile([128, NCONF * 2], I16)
        gidx2_s = cpool.tile([128, NCONF], I16)
        nc.sync.dma_start(w32t_s[:], w32t[:])
        nc.sync.dma_start(ident_s[:], ident[:])
        nc.sync.dma_start(cirep_s[:], cirep[:])
        nc.sync.dma_start(gidx1_s[:], gidx1[:])
        nc.sync.dma_start(gidx2_s[:], gidx2[:])

        out_sb = outp.tile([128, NBT], F32)

        ao3 = ao.rearrange("(t p) n -> t p n", p=128)
        # mscr: [32m, BC*32(b,e)] fp16 -- M^T layout
        mscr_r = mscr.rearrange("m (b e) -> b m e", e=NE)   # [BC, 32, 32]

        def lu_chunk_ops(sub_t, P_t, L_t, rec_t, ks_gps):
            """Per-step op emitter for one 32-cfg chunk.

            sub layout per partition: [CL, 16j, 16i] fp32 (transposed
            submatrix; det unchanged).  In-place elimination on (j, i)."""
            S4 = sub_t[:].rearrange("p (c j i) -> p c j i", j=K, i=K)
            L3 = L_t[:].rearrange("p (c i) -> p c i", c=CL)
            P4 = P_t[:].rearrange("p (c j i) -> p c j i", j=K - 1, i=K - 1)

            def step(k):
                r = K - 1 - k
                piv = S4[:, :, k, k]
                nc.vector.reciprocal(rec_t[:], piv)
                nc.vector.tensor_scalar(
                    rec_t[:], rec_t[:], -RCLAMP, RCLAMP,
                    op0=OP.max, op1=OP.min)
                # L[c,i] = col * rec  (col = S4[:, :, k, k+1:], i-dim)
                col = S4[:, :, k, k + 1:]
                Lv = L3[:, :, :r]
                nc.vector.tensor_tensor(
                    Lv, col,
                    rec_t[:].unsqueeze(2).broadcast_to([128, CL, r]),
                    op=OP.mult)
                # P[c,j,i] = row[c,j] x L[c,i]  (row = S4[:, :, k+1:, k])
                row = S4[:, :, k + 1:, k]
                Pv = P4[:, :, :r, :r]
                nc.vector.tensor_tensor(
                    Pv,
                    row.unsqueeze(3).broadcast_to([128, CL, r, r]),
                    Lv.unsqueeze(2).broadcast_to([128, CL, r, r]),
                    op=OP.mult)
                # S -= P
                Sv = S4[:, :, k + 1:, k + 1:]
                eng = nc.gpsimd if k in ks_gps else nc.vector
                eng.tensor_tensor(Sv, Sv, Pv, op=OP.subtract)
            return step

        def lu_finish(sub_t, dets_t, ch):
            """Trailing 2x2 det + product tree into dets_t[:, ch*CL:...]."""
            S4 = sub_t[:].rearrange("p (c j i) -> p c j i", j=K, i=K)
            t8 = sm.tile([128, CL * 8], F32, tag=f"t8{ch % 2}")
            t8v = t8[:].rearrange("p (c x) -> p c x", c=CL)
            # 7 diag pairs (k=0..13)
            d = sub_t[:]
            nc.vector.tensor_tensor(
                t8v[:, :, :7],
                bass.AP(d.tensor, d.offset,
                        [[int(d.ap[0][0]), 128], [K * K, CL], [34, 7]]),
                bass.AP(d.tensor, d.offset + 17,
                        [[int(d.ap[0][0]), 128], [K * K, CL], [34, 7]]),
                op=OP.mult)
            # det2 of trailing 2x2: S[14,14]*S[15,15] - S[14,15]*S[15,14]
            m1 = sm.tile([128, CL], F32, tag=f"m1{ch % 2}")
            nc.vector.tensor_tensor(
                m1[:], S4[:, :, K - 2, K - 2], S4[:, :, K - 1, K - 1],
                op=OP.mult)
            m2 = sm.tile([128, CL], F32, tag=f"m2{ch % 2}")
            nc.vector.tensor_tensor(
                m2[:], S4[:, :, K - 2, K - 1], S4[:, :, K - 1, K - 2],
                op=OP.mult)
            nc.vector.tensor_tensor(t8v[:, :, 7], m1[:], m2[:],
                                    op=OP.subtract)
            # tree 8 -> 4 -> 2 -> 1
            t4 = sm.tile([128, CL * 4], F32, tag=f"t4{ch % 2}")
            nc.vector.tensor_tensor(
                t4[:].rearrange("p (c x) -> p c x", c=CL),
                bass.AP(t8[:].tensor, t8[:].offset,
                        [[int(t8[:].ap[0][0]), 128], [8, CL], [2, 4]]),
                bass.AP(t8[:].tensor, t8[:].offset + 1,
                        [[int(t8[:].ap[0][0]), 128], [8, CL], [2, 4]]),
                op=OP.mult)
            t2 = sm.tile([128, CL * 2], F32, tag=f"t2{ch % 2}")
            nc.vector.tensor_tensor(
                t2[:].rearrange("p (c x) -> p c x", c=CL),
                bass.AP(t4[:].tensor, t4[:].offset,
                        [[int(t4[:].ap[0][0]), 128], [4, CL], [2, 2]]),
                bass.AP(t4[:].tensor, t4[:].offset + 1,
                        [[int(t4[:].ap[0][0]), 128], [4, CL], [2, 2]]),
                op=OP.mult)
            nc.vector.tensor_tensor(
                dets_t[:, ch * CL:(ch + 1) * CL],
                bass.AP(t2[:].tensor, t2[:].offset,
                        [[int(t2[:].ap[0][0]), 128], [2, CL]]),
                bass.AP(t2[:].tensor, t2[:].offset + 1,
                        [[int(t2[:].ap[0][0]), 128], [2, CL]]),
                op=OP.mult)

        def phase1(bt):
            """M^T = W32 @ ao^T, written to mscr in fp16."""
            for t in range(32):
                nat_t = nat.tile([128, 128], F32)
                nc.sync.dma_start(nat_t[:], ao3[bt * 32 + t])
                ps = tp_ps.tile([128, 128], F32)
                nc.tensor.transpose(ps[:], nat_t[:], ident_s[:])
                aot_t = aot.tile([128, 128], F32)
                nc.scalar.copy(aot_t[:], ps[:])
                mp = m_ps.tile([NE, 128], F32)
                nc.tensor.matmul(
                    mp[:], w32t_s[:], aot_t[:], start=True, stop=True)
                msb_s = nat.tile([NE, 128], F16, tag="mstage")
                nc.scalar.copy(msb_s[:], mp[:])
                nc.scalar.dma_start(
                    mscr[:, (bt * 128 + t * 4) * NE:
                         (bt * 128 + t * 4 + 4) * NE],
                    msb_s[:])

        msb_tiles = {}

        def get_msb(bt):
            if bt not in msb_tiles:
                t = msb.tile([128, NE * NE], F16, tag=f"m{bt % 2}")
                nc.sync.dma_start(t[:], mscr_r[bt * 128:(bt + 1) * 128])
                msb_tiles[bt] = t
            return msb_tiles[bt]

        sub_tiles = {}

        def gather_piece(bt, ch, g):
            """Emit one c8-chunk gather (g1 -> transpose -> g2)."""
            key = (bt, ch)
            if key not in sub_tiles:
                ci = bt * NCH + ch
                sub_tiles[key] = subp.tile(
                    [128, CL * K * K], F32, name=f"sub{ci % 3}",
                    tag=f"s{ci % 3}")
            sub_t = sub_tiles[key]
            msb_t = get_msb(bt)
            cg = ch * NG + g               # c8-chunk id within btile
            r_t = rp.tile([128, CG * K * NE], F16)
            nc.gpsimd.ap_gather(
                r_t[:], msb_t[:],
                gidx1_s[:, cg * 16:(cg + 1) * 16],
                channels=128, num_elems=NE * 2, d=16,
                num_idxs=CG * K * 2)
            # transpose + cast fp16 -> fp32: [c,i,m] -> [c,m,i]
            # walk order (c,i,m): contiguous reads, strided writes
            rt_t = rtp.tile([128, CG * K * NE], F32)
            rt_dst = bass.AP(
                rt_t[:].tensor, rt_t[:].offset,
                [[int(rt_t[:].ap[0][0]), 128],
                 [K * NE, CG], [1, K], [K, NE]])
            r_src = bass.AP(
                r_t[:].tensor, r_t[:].offset,
                [[int(r_t[:].ap[0][0]), 128],
                 [K * NE, CG], [NE, K], [1, NE]])
            nc.scalar.copy(rt_dst, r_src)
            nc.gpsimd.ap_gather(
                sub_t[:, g * CG * K * K:(g + 1) * CG * K * K],
                rt_t[:],
                gidx2_s[:, cg * 8:(cg + 1) * 8],
                channels=128, num_elems=CG * NE, d=16,
                num_idxs=CG * K)

        # flat chunk order over the whole program; gathers for chunk
        # pair p+1 are emitted inside pair p's k-loop (slots k=6..13)
        all_chunks = [(bt, ch) for bt in range(NBT) for ch in range(NCH)]
        pieces = [(bt, ch, g) for (bt, ch) in all_chunks for g in range(NG)]
        pos = 0                            # next piece to emit

        phase1(0)
        # prime the pipeline: chunks 0 and 1 of btile 0
        while pos < 2 * NG:
            bt_, ch_, g_ = pieces[pos]
            gather_piece(bt_, ch_, g_)
            pos += 1

        for bt in range(NBT):
            if bt + 1 < NBT:
                phase1(bt + 1)
            dets_t = dets.tile([128, NCONF], F32)
            for chp in range(NCH // 2):
                chA, chB = chp * 2, chp * 2 + 1
                subA = sub_tiles[(bt, chA)]
                subB = sub_tiles[(bt, chB)]
                PA = pb.tile([128, CL * (K - 1) * (K - 1)], F32, tag="pA")
                PB = pb.tile([128, CL * (K - 1) * (K - 1)], F32, tag="pB")
                LA = lb.tile([128, CL * (K - 1)], F32, tag="lA")
                LB = lb.tile([128, CL * (K - 1)], F32, tag="lB")
                recA = sm.tile([128, CL], F32, tag="recA")
                recB = sm.tile([128, CL], F32, tag="recB")
                stepA = lu_chunk_ops(subA, PA, LA, recA, KS_GPS_A)
                stepB = lu_chunk_ops(subB, PB, LB, recB, KS_GPS_B)
                npc = min(2 * NG, len(pieces) - pos)   # pieces this pair
                for k in range(K - 2):
                    stepA(k)
                    stepB(k)
                    # emit next-pair gathers in the tail of the k-loop
                    if k >= (K - 2) - npc:
                        bt_, ch_, g_ = pieces[pos]
                        gather_piece(bt_, ch_, g_)
                        pos += 1
                lu_finish(subA, dets_t, chA)
                lu_finish(subB, dets_t, chB)
                del sub_tiles[(bt, chA)]
                del sub_tiles[(bt, chB)]

            wd = sm.tile([128, NCONF], F32, tag="wd")
            nc.vector.tensor_tensor(wd[:], dets_t[:], cirep_s[:], op=OP.mult)
            nc.vector.tensor_reduce(
                out_sb[:, bt:bt + 1], wd[:], axis=AX.X, op=OP.add)
            msb_tiles.pop(bt, None)

        nc.sync.dma_start(out[:], out_sb[:])


def build(BCc: int):
    nc = bacc.Bacc("TRN2", target_bir_lowering=False, debug=False)
    aps = {}
    aps["ao"] = nc.dram_tensor(
        "ao", [BCc * NE, NAO], F32, kind="ExternalInput").ap()
    aps["w32t"] = nc.dram_tensor(
        "w32t", [NAO, NE], F32, kind="ExternalInput").ap()
    aps["ident"] = nc.dram_tensor(
        "ident", [128, 128], F32, kind="ExternalInput").ap()
    aps["cirep"] = nc.dram_tensor(
        "cirep", [128, NCONF], F32, kind="ExternalInput").ap()
    aps["gidx1"] = nc.dram_tensor(
        "gidx1", [128, NCONF * 2], I16, kind="ExternalInput").ap()
    aps["gidx2"] = nc.dram_tensor(
        "gidx2", [128, NCONF], I16, kind="ExternalInput").ap()
    aps["mscr"] = nc.dram_tensor("mscr", [NE, BCc * NE], F16).ap()
    aps["out"] = nc.dram_tensor(
        "out", [128, BCc // 128], F32, kind="ExternalOutput").ap()

    with tile.TileContext(nc) as tc:
        emit_program(nc, tc, aps, BCc)
    nc.compile()
    return nc


def host_inputs(ao_shard, mo_weight, ci_weight, configs):
    BCc = ao_shard.shape[0]
    w32 = mo_weight[:NE, :]
    return {
        "ao": np.ascontiguousarray(
            ao_shard.reshape(BCc * NE, NAO)).astype(np.float32),
        "w32t": np.ascontiguousarray(w32.T).astype(np.float32),
        "ident": np.eye(128, dtype=np.float32),
        "cirep": np.ascontiguousarray(
            np.tile(ci_weight.astype(np.float32), (128, 1))),
        "gidx1": build_gidx1(configs),
        "gidx2": build_gidx2(configs),
    }


_CACHE: dict = {}


def _get_program():
    key = ("prog", BC, CL, KS_GPS_A, KS_GPS_B)
    if key not in _CACHE:
        _CACHE[key] = build(BC)
    return _CACHE[key]


def kernel(ao, mo_weight, ci_weight, configs):
    ao = np.asarray(ao, dtype=np.float32)
    mo_weight = np.asarray(mo_weight, dtype=np.float32)
    ci_weight = np.asarray(ci_weight, dtype=np.float32)
    configs = np.asarray(configs, dtype=np.int32)
    assert ao.shape == (B, NE, NAO)

    nc = _get_program()
    in_maps = [
        host_inputs(ao[c * BC:(c + 1) * BC], mo_weight, ci_weight, configs)
        for c in range(NCORES)
    ]
    res = run_bass_kernel_spmd(nc, in_maps, core_ids=list(range(NCORES)))
    outs = []
    for c in range(NCORES):
        o = np.asarray(res.results[c]["out"])      # [128, NBT]
        outs.append(o.T.reshape(-1))               # b = bt*128 + p
    return np.concatenate(outs).astype(np.float32)[:, None]


def ref_algo(ao_shard, mo_weight, ci_weight, configs):
    """Numpy replica of the on-device algorithm (dev checking only)."""
    M = np.einsum("ben,mn->bem", ao_shard, mo_weight[:NE]).astype(np.float32)
    M = M.astype(np.float16).astype(np.float32)
    sub = M[:, configs[:, :, None], configs[:, None, :]].astype(np.float32)
    subT = np.swapaxes(sub, -1, -2)
    Bs = subT.shape[0]
    A = subT.reshape(-1, K, K).copy()
    rcl = np.float32(RCLAMP)
    dets = np.ones(A.shape[0], np.float32)
    for k in range(K - 2):
        piv = A[:, k, k].copy()
        with np.errstate(divide="ignore"):
            rec = (np.float32(1.0) / piv).astype(np.float32)
        rec = np.clip(rec, -rcl, rcl)
        L = (A[:, k, k + 1:] * rec[:, None]).astype(np.float32)
        A[:, k + 1:, k + 1:] -= (
            A[:, k + 1:, k][:, :, None] * L[:, None, :]).astype(np.float32)
        dets = (dets * piv).astype(np.float32)
    det2 = (A[:, K - 2, K - 2] * A[:, K - 1, K - 1]
            - A[:, K - 2, K - 1] * A[:, K - 1, K - 2]).astype(np.float32)
    dets = (dets * det2).astype(np.float32)
    dets_ = dets.reshape(Bs, NCONF)
    return (dets_ @ ci_weight.T.astype(np.float32)).astype(np.float32)


# revision 14
# speedup vs baseline: 1.1901x; 1.1901x over previous
"""Trainium2 Bass kernel for nn_NEURAL_PYSCF_WF (neural wavefunction).

reference:
  mo   = einsum('ben,mn->bem', ao, mo_weight)          # [B, 32, 128]
  sub  = mo[:, cfg[:,:,None], cfg[:,None,:]]           # [B, 128, 16, 16]
  dets = det(sub)                                      # [B, 128]
  out  = dets @ ci_weight.T                            # [B, 1]

Config indices are < 32, so only mo[:, :, :32] matters.

Strategy (8 NeuronCores, data-parallel over B=8192). Per core (1024 rows):
  phase 1: ao tiles -> PE transpose -> matmul (W32T stationary) ->
           M^T [m,(b,e)] in fp16 -> DRAM scratch; reload per 128-row
           b-tile as M [128b, 1024(e,m)] fp16.
  phase 2 per b-tile (128 walkers in partitions):
    per c8 sub-chunk: gpsimd ap_gather (d=16, fp16) config rows ->
      R [c,i,m32] fp16; ScalarE strided copy-transpose+cast ->
      Rt [c,m32,i16] fp32; gpsimd ap_gather (d=16, fp32) config cols ->
      sub [c,j,i] fp32 (det(A^T)==det(A)).
    LU: pivot-free elimination over chunks of 32 configs, two chunks
      interleaved so cross-engine gaps are filled.  Reciprocal clamped
      to +-1e6.  The S -= L*row update for the biggest steps
      (k in KS_GPS) runs on GPSIMD in parallel with DVE.
      Trailing 2x2 determinant in closed form; product tree in fp32.
  out[b] = sum_c ci[c] * det[b, c]  (TT mult + reduce).
"""

from contextlib import ExitStack

import numpy as np

import concourse.bass as bass
import concourse.bacc as bacc
import concourse.mybir as mybir
import concourse.tile as tile
from concourse.bass_utils import run_bass_kernel_spmd

F32 = mybir.dt.float32
F16 = mybir.dt.float16
I16 = mybir.dt.int16
AX = mybir.AxisListType
OP = mybir.AluOpType

B = 8192
NE = 32      # electrons (and the max config index)
NAO = 128
K = 16       # config size
NCONF = 128
NCORES = 8
BC = B // NCORES
RCLAMP = 1e6
CG = 8       # configs per gather chunk
CL = 32      # configs per LU chunk
# elimination steps whose S-update runs on GPSIMD, per stream (A, B).
# Empty: gpsimd TT thrashes the ext-isa IRAM against ap_gather (~6us
# reload per alternation) and stalls the DVE dependency chain.
KS_GPS_A = ()
KS_GPS_B = ()


def wrap_idx(idx: np.ndarray) -> np.ndarray:
    """Wrap a flat index list into ap_gather's [128, n/16] layout."""
    n = idx.shape[0]
    assert n % 16 == 0
    w = idx.reshape(n // 16, 16).T.astype(np.int16)
    return np.tile(w, (8, 1))


def build_gidx1(cfg: np.ndarray) -> np.ndarray:
    """Stage-1 indices per c8 chunk: (c,i,h) -> block cfg[c,i]*2+h."""
    cols = []
    for ch in range(NCONF // CG):
        sl = cfg[ch * CG:(ch + 1) * CG]                    # [CG, 16]
        idx = (sl[:, :, None].astype(np.int64) * 2
               + np.arange(2)[None, None, :]).reshape(-1)  # CG*K*2 = 256
        cols.append(wrap_idx(idx))                         # [128, 16]
    return np.concatenate(cols, axis=1)                    # [128, 256]


def build_gidx2(cfg: np.ndarray) -> np.ndarray:
    """Stage-2 indices per c8 chunk: (c_local, j) -> c_local*32 + cfg[c,j]."""
    cols = []
    for ch in range(NCONF // CG):
        sl = cfg[ch * CG:(ch + 1) * CG]                    # [CG, 16]
        idx = (np.arange(CG)[:, None] * NE + sl).reshape(-1)   # CG*K = 128
        cols.append(wrap_idx(idx))                         # [128, 8]
    return np.concatenate(cols, axis=1)                    # [128, 128]


def emit_program(nc, tc, aps, BCc: int):
    ctx = ExitStack()
    NBT = BCc // 128
    NCH = NCONF // CL          # LU chunks per btile (4)
    NG = CL // CG              # gather chunks per LU chunk (4)
    ao, w32t, ident, cirep, gidx1, gidx2, mscr, out = (
        aps["ao"], aps["w32t"], aps["ident"], aps["cirep"], aps["gidx1"],
        aps["gidx2"], aps["mscr"], aps["out"])

    with ctx:
        cpool = ctx.enter_context(tc.tile_pool(name="consts", bufs=1))
        nat = ctx.enter_context(tc.tile_pool(name="nat", bufs=4))
        tp_ps = ctx.enter_context(
            tc.tile_pool(name="tp_ps", bufs=3, space="PSUM"))
        aot = ctx.enter_context(tc.tile_pool(name="aot", bufs=2))
        m_ps = ctx.enter_context(
            tc.tile_pool(name="m_ps", bufs=3, space="PSUM"))
        msb = ctx.enter_context(tc.tile_pool(name="msb", bufs=1))
        rp = ctx.enter_context(tc.tile_pool(name="rp", bufs=1))
        rtp = ctx.enter_context(tc.tile_pool(name="rtp", bufs=1))
        subp = ctx.enter_context(tc.tile_pool(name="subp", bufs=1))
        lb = ctx.enter_context(tc.tile_pool(name="lb", bufs=1))
        pb = ctx.enter_context(tc.tile_pool(name="pb", bufs=1))
        sm = ctx.enter_context(tc.tile_pool(name="sm", bufs=2))
        dets = ctx.enter_context(tc.tile_pool(name="dets", bufs=2))
        outp = ctx.enter_context(tc.tile_pool(name="outp", bufs=1))

        w32t_s = cpool.tile([128, NE], F32)
        ident_s = cpool.tile([128, 128], F32)
        cirep_s = cpool.tile([128, NCONF], F32)
        gidx1_s = cpool.tile([128, NCONF * 2], I16)
        gidx2_s = cpool.tile([128, NCONF], I16)
        nc.sync.dma_start(w32t_s[:], w32t[:])
        nc.sync.dma_start(ident_s[:], ident[:])
        nc.sync.dma_start(cirep_s[:], cirep[:])
        nc.sync.dma_start(gidx1_s[:], gidx1[:])
        nc.sync.dma_start(gidx2_s[:], gidx2[:])

        out_sb = outp.tile([128, NBT], F32)

        ao3 = ao.rearrange("(t p) n -> t p n", p=128)
        # mscr: [32m, BC*32(b,e)] fp16 -- M^T layout
        mscr_r = mscr.rearrange("m (b e) -> b m e", e=NE)   # [BC, 32, 32]

        def lu_chunk_ops(sub_t, P_t, L_t, rec_t, ks_gps):
            """Per-step op emitter for one 32-cfg chunk.

            sub layout per partition: [CL, 16j, 16i] fp32 (transposed
            submatrix; det unchanged).  In-place elimination on (j, i)."""
            S4 = sub_t[:].rearrange("p (c j i) -> p c j i", j=K, i=K)
            L3 = L_t[:].rearrange("p (c i) -> p c i", c=CL)
            P4 = P_t[:].rearrange("p (c j i) -> p c j i", j=K - 1, i=K - 1)

            def step(k):
                r = K - 1 - k
                piv = S4[:, :, k, k]
                nc.vector.reciprocal(rec_t[:], piv)
                nc.vector.tensor_scalar(
                    rec_t[:], rec_t[:], -RCLAMP, RCLAMP,
                    op0=OP.max, op1=OP.min)
                # L[c,i] = col * rec  (col = S4[:, :, k, k+1:], i-dim)
                col = S4[:, :, k, k + 1:]
                Lv = L3[:, :, :r]
                nc.vector.tensor_tensor(
                    Lv, col,
                    rec_t[:].unsqueeze(2).broadcast_to([128, CL, r]),
                    op=OP.mult)
                # P[c,j,i] = row[c,j] x L[c,i]  (row = S4[:, :, k+1:, k])
                row = S4[:, :, k + 1:, k]
                Pv = P4[:, :, :r, :r]
                nc.vector.tensor_tensor(
                    Pv,
                    row.unsqueeze(3).broadcast_to([128, CL, r, r]),
                    Lv.unsqueeze(2).broadcast_to([128, CL, r, r]),
                    op=OP.mult)
                # S -= P
                Sv = S4[:, :, k + 1:, k + 1:]
                eng = nc.gpsimd if k in ks_gps else nc.vector
                eng.tensor_tensor(Sv, Sv, Pv, op=OP.subtract)
            return step

        def lu_finish(sub_t, dets_t, ch):
            """Trailing 2x2 det + product tree into dets_t[:, ch*CL:...]."""
            S4 = sub_t[:].rearrange("p (c j i) -> p c j i", j=K, i=K)
            t8 = sm.tile([128, CL * 8], F32, tag=f"t8{ch % 2}")
            t8v = t8[:].rearrange("p (c x) -> p c x", c=CL)
            # 7 diag pairs (k=0..13)
            d = sub_t[:]
            nc.vector.tensor_tensor(
                t8v[:, :, :7],
                bass.AP(d.tensor, d.offset,
                        [[int(d.ap[0][0]), 128], [K * K, CL], [34, 7]]),
                bass.AP(d.tensor, d.offset + 17,
                        [[int(d.ap[0][0]), 128], [K * K, CL], [34, 7]]),
                op=OP.mult)
            # det2 of trailing 2x2: S[14,14]*S[15,15] - S[14,15]*S[15,14]
            m1 = sm.tile([128, CL], F32, tag=f"m1{ch % 2}")
            nc.vector.tensor_tensor(
                m1[:], S4[:, :, K - 2, K - 2], S4[:, :, K - 1, K - 1],
                op=OP.mult)
            m2 = sm.tile([128, CL], F32, tag=f"m2{ch % 2}")
            nc.vector.tensor_tensor(
                m2[:], S4[:, :, K - 2, K - 1], S4[:, :, K - 1, K - 2],
                op=OP.mult)
            nc.vector.tensor_tensor(t8v[:, :, 7], m1[:], m2[:],
                                    op=OP.subtract)
            # tree 8 -> 4 -> 2 -> 1
            t4 = sm.tile([128, CL * 4], F32, tag=f"t4{ch % 2}")
            nc.vector.tensor_tensor(
                t4[:].rearrange("p (c x) -> p c x", c=CL),
                bass.AP(t8[:].tensor, t8[:].offset,
                        [[int(t8[:].ap[0][0]), 128], [8, CL], [2, 4]]),
                bass.AP(t8[:].tensor, t8[:].offset + 1,
                        [[int(t8[:].ap[0][0]), 128], [8, CL], [2, 4]]),
                op=OP.mult)
            t2 = sm.tile([128, CL * 2], F32, tag=f"t2{ch % 2}")
            nc.vector.tensor_tensor(
                t2[:].rearrange("p (c x) -> p c x", c=CL),
                bass.AP(t4[:].tensor, t4[:].offset,
                        [[int(t4[:].ap[0][0]), 128], [4, CL], [2, 2]]),
                bass.AP(t4[:].tensor, t4[:].offset + 1,
                        [[int(t4[:].ap[0][0]), 128], [4, CL], [2, 2]]),
                op=OP.mult)
            nc.vector.tensor_tensor(
                dets_t[:, ch * CL:(ch + 1) * CL],
                bass.AP(t2[:].tensor, t2[:].offset,
                        [[int(t2[:].ap[0][0]), 128], [2, CL]]),
                bass.AP(t2[:].tensor, t2[:].offset + 1,
                        [[int(t2[:].ap[0][0]), 128], [2, CL]]),
                op=OP.mult)

        def phase1(bt):
            """M^T = W32 @ ao^T, written to mscr in fp16."""
            for t in range(32):
                nat_t = nat.tile([128, 128], F32)
                nc.sync.dma_start(nat_t[:], ao3[bt * 32 + t])
                ps = tp_ps.tile([128, 128], F32)
                nc.tensor.transpose(ps[:], nat_t[:], ident_s[:])
                aot_t = aot.tile([128, 128], F32)
                nc.scalar.copy(aot_t[:], ps[:])
                mp = m_ps.tile([NE, 128], F32)
                nc.tensor.matmul(
                    mp[:], w32t_s[:], aot_t[:], start=True, stop=True)
                msb_s = nat.tile([NE, 128], F16, tag="mstage")
                nc.scalar.copy(msb_s[:], mp[:])
                nc.scalar.dma_start(
                    mscr[:, (bt * 128 + t * 4) * NE:
                         (bt * 128 + t * 4 + 4) * NE],
                    msb_s[:])

        msb_tiles = {}

        def get_msb(bt):
            if bt not in msb_tiles:
                t = msb.tile([128, NE * NE], F16, tag=f"m{bt % 2}")
                nc.sync.dma_start(t[:], mscr_r[bt * 128:(bt + 1) * 128])
                msb_tiles[bt] = t
            return msb_tiles[bt]

        sub_tiles = {}

        def gather_piece(bt, ch, g):
            """Emit one c8-chunk gather (g1 -> transpose -> g2)."""
            key = (bt, ch)
            if key not in sub_tiles:
                ci = bt * NCH + ch
                sub_tiles[key] = subp.tile(
                    [128, CL * K * K], F32, name=f"sub{ci % 3}",
                    tag=f"s{ci % 3}")
            sub_t = sub_tiles[key]
            msb_t = get_msb(bt)
            cg = ch * NG + g               # c8-chunk id within btile
            r_t = rp.tile([128, CG * K * NE], F16)
            nc.gpsimd.ap_gather(
                r_t[:], msb_t[:],
                gidx1_s[:, cg * 16:(cg + 1) * 16],
                channels=128, num_elems=NE * 2, d=16,
                num_idxs=CG * K * 2)
            # transpose + cast fp16 -> fp32: [c,i,m] -> [c,m,i]
            # walk order (c,i,m): contiguous reads, strided writes
            rt_t = rtp.tile([128, CG * K * NE], F32)
            rt_dst = bass.AP(
                rt_t[:].tensor, rt_t[:].offset,
                [[int(rt_t[:].ap[0][0]), 128],
                 [K * NE, CG], [1, K], [K, NE]])
            r_src = bass.AP(
                r_t[:].tensor, r_t[:].offset,
                [[int(r_t[:].ap[0][0]), 128],
                 [K * NE, CG], [NE, K], [1, NE]])
            nc.scalar.copy(rt_dst, r_src)
            nc.gpsimd.ap_gather(
                sub_t[:, g * CG * K * K:(g + 1) * CG * K * K],
                rt_t[:],
                gidx2_s[:, cg * 8:(cg + 1) * 8],
                channels=128, num_elems=CG * NE, d=16,
                num_idxs=CG * K)

        # flat chunk order over the whole program; gathers for chunk
        # pair p+1 are emitted inside pair p's k-loop (slots k=6..13)
        all_chunks = [(bt, ch) for bt in range(NBT) for ch in range(NCH)]
        pieces = [(bt, ch, g) for (bt, ch) in all_chunks for g in range(NG)]
        pos = 0                            # next piece to emit

        phase1(0)
        # prime the pipeline: chunks 0 and 1 of btile 0
        while pos < 2 * NG:
            bt_, ch_, g_ = pieces[pos]
            gather_piece(bt_, ch_, g_)
            pos += 1

        for bt in range(NBT):
            if bt + 1 < NBT:
                phase1(bt + 1)
            dets_t = dets.tile([128, NCONF], F32)
            for chp in range(NCH // 2):
                chA, chB = chp * 2, chp * 2 + 1
                subA = sub_tiles[(bt, chA)]
                subB = sub_tiles[(bt, chB)]
                PA = pb.tile([128, CL * (K - 1) * (K - 1)], F32, tag="pA")
                PB = pb.tile([128, CL * (K - 1) * (K - 1)], F32, tag="pB")
                LA = lb.tile([128, CL * (K - 1)], F32, tag="lA")
                LB = lb.tile([128, CL * (K - 1)], F32, tag="lB")
                recA = sm.tile([128, CL], F32, tag="recA")
                recB = sm.tile([128, CL], F32, tag="recB")
                stepA = lu_chunk_ops(subA, PA, LA, recA, KS_GPS_A)
                stepB = lu_chunk_ops(subB, PB, LB, recB, KS_GPS_B)
                npc = min(2 * NG, len(pieces) - pos)   # pieces this pair
                for k in range(K - 2):
                    stepA(k)
                    stepB(k)
                    # emit next-pair gathers in the tail of the k-loop
                    if k >= (K - 2) - npc:
                        bt_, ch_, g_ = pieces[pos]
                        gather_piece(bt_, ch_, g_)
                        pos += 1
                lu_finish(subA, dets_t, chA)
                lu_finish(subB, dets_t, chB)
                del sub_tiles[(bt, chA)]
                del sub_tiles[(bt, chB)]

            wd = sm.tile([128, NCONF], F32, tag="wd")
            nc.vector.tensor_tensor(wd[:], dets_t[:], cirep_s[:], op=OP.mult)
            nc.vector.tensor_reduce(
                out_sb[:, bt:bt + 1], wd[:], axis=AX.X, op=OP.add)
            msb_tiles.pop(bt, None)

        nc.sync.dma_start(out[:], out_sb[:])


def build(BCc: int):
    nc = bacc.Bacc("TRN2", target_bir_lowering=False, debug=False)
    aps = {}
    aps["ao"] = nc.dram_tensor(
        "ao", [BCc * NE, NAO], F32, kind="ExternalInput").ap()
    aps["w32t"] = nc.dram_tensor(
        "w32t", [NAO, NE], F32, kind="ExternalInput").ap()
    aps["ident"] = nc.dram_tensor(
        "ident", [128, 128], F32, kind="ExternalInput").ap()
    aps["cirep"] = nc.dram_tensor(
        "cirep", [128, NCONF], F32, kind="ExternalInput").ap()
    aps["gidx1"] = nc.dram_tensor(
        "gidx1", [128, NCONF * 2], I16, kind="ExternalInput").ap()
    aps["gidx2"] = nc.dram_tensor(
        "gidx2", [128, NCONF], I16, kind="ExternalInput").ap()
    aps["mscr"] = nc.dram_tensor("mscr", [NE, BCc * NE], F16).ap()
    aps["out"] = nc.dram_tensor(
        "out", [128, BCc // 128], F32, kind="ExternalOutput").ap()

    with tile.TileContext(nc) as tc:
        emit_program(nc, tc, aps, BCc)
    nc.compile()
    return nc


def host_inputs(ao_shard, mo_weight, ci_weight, configs):
    BCc = ao_shard.shape[0]
    w32 = mo_weight[:NE, :]
    return {
        "ao": np.ascontiguousarray(
            ao_shard.reshape(BCc * NE, NAO)).astype(np.float32),
        "w32t": np.ascontiguousarray(w32.T).astype(np.float32),
        "ident": np.eye(128, dtype=np.float32),
        "cirep": np.ascontiguousarray(
            np.tile(ci_weight.astype(np.float32), (128, 1))),
        "gidx1": build_gidx1(configs),
        "gidx2": build_gidx2(configs),
    }


_CACHE: dict = {}


def _get_program():
    key = ("prog", BC, CL, KS_GPS_A, KS_GPS_B)
    if key not in _CACHE:
        _CACHE[key] = build(BC)
    return _CACHE[key]


def kernel(ao, mo_weight, ci_weight, configs):
    ao = np.asarray(ao, dtype=np.float32)
    mo_weight = np.asarray(mo_weight, dtype=np.float32)
    ci_weight = np.asarray(ci_weight, dtype=np.float32)
    configs = np.asarray(configs, dtype=np.int32)
    assert ao.shape == (B, NE, NAO)

    nc = _get_program()
    in_maps = [
        host_inputs(ao[c * BC:(c + 1) * BC], mo_weight, ci_weight, configs)
        for c in range(NCORES)
    ]
    res = run_bass_kernel_spmd(nc, in_maps, core_ids=list(range(NCORES)))
    outs = []
    for c in range(NCORES):
        o = np.asarray(res.results[c]["out"])      # [128, NBT]
        outs.append(o.T.reshape(-1))               # b = bt*128 + p
    return np.concatenate(outs).astype(np.float32)[:, None]


def ref_algo(ao_shard, mo_weight, ci_weight, configs):
    """Numpy replica of the on-device algorithm (dev checking only)."""
    M = np.einsum("ben,mn->bem", ao_shard, mo_weight[:NE]).astype(np.float32)
    M = M.astype(np.float16).astype(np.float32)
    sub = M[:, configs[:, :, None], configs[:, None, :]].astype(np.float32)
    subT = np.swapaxes(sub, -1, -2)
    Bs = subT.shape[0]
    A = subT.reshape(-1, K, K).copy()
    rcl = np.float32(RCLAMP)
    dets = np.ones(A.shape[0], np.float32)
    for k in range(K - 2):
        piv = A[:, k, k].copy()
        with np.errstate(divide="ignore"):
            rec = (np.float32(1.0) / piv).astype(np.float32)
        rec = np.clip(rec, -rcl, rcl)
        L = (A[:, k, k + 1:] * rec[:, None]).astype(np.float32)
        A[:, k + 1:, k + 1:] -= (
            A[:, k + 1:, k][:, :, None] * L[:, None, :]).astype(np.float32)
        dets = (dets * piv).astype(np.float32)
    det2 = (A[:, K - 2, K - 2] * A[:, K - 1, K - 1]
            - A[:, K - 2, K - 1] * A[:, K - 1, K - 2]).astype(np.float32)
    dets = (dets * det2).astype(np.float32)
    dets_ = dets.reshape(Bs, NCONF)
    return (dets_ @ ci_weight.T.astype(np.float32)).astype(np.float32)


# revision 16
# speedup vs baseline: 1.3566x; 1.1399x over previous
"""Trainium2 Bass kernel for nn_NEURAL_PYSCF_WF (neural wavefunction).

reference:
  mo   = einsum('ben,mn->bem', ao, mo_weight)          # [B, 32, 128]
  sub  = mo[:, cfg[:,:,None], cfg[:,None,:]]           # [B, 128, 16, 16]
  dets = det(sub)                                      # [B, 128]
  out  = dets @ ci_weight.T                            # [B, 1]

Config indices are < 32, so only mo[:, :, :32] matters.

Strategy (8 NeuronCores, data-parallel over B=8192). Per core (1024 rows):
  phase 1: ao tiles -> PE transpose -> matmul (W32T stationary) ->
           M^T [m,(b,e)] in fp16 -> DRAM scratch; reload per 128-row
           b-tile as M [128b, 1024(e,m)] fp16.
  phase 2 per b-tile (128 walkers in partitions):
    per c8 sub-chunk: gpsimd ap_gather (d=16, fp16) config rows ->
      R [c,i,m32] fp16; ScalarE strided copy-transpose+cast ->
      Rt [c,m32,i16] fp32; gpsimd ap_gather (d=16, fp32) config cols ->
      sub [c,j,i] fp32 (det(A^T)==det(A)).
    LU: pivot-free elimination over chunks of 32 configs, two chunks
      interleaved so cross-engine gaps are filled.  Reciprocal clamped
      to +-1e6.  The S -= L*row update for the biggest steps
      (k in KS_GPS) runs on GPSIMD in parallel with DVE.
      Trailing 2x2 determinant in closed form; product tree in fp32.
  out[b] = sum_c ci[c] * det[b, c]  (TT mult + reduce).
"""

from contextlib import ExitStack

import numpy as np

import concourse.bass as bass
import concourse.bacc as bacc
import concourse.mybir as mybir
import concourse.tile as tile
from concourse.bass_utils import run_bass_kernel_spmd

F32 = mybir.dt.float32
F16 = mybir.dt.float16
I16 = mybir.dt.int16
AX = mybir.AxisListType
OP = mybir.AluOpType

B = 8192
NE = 32      # electrons (and the max config index)
NAO = 128
K = 16       # config size
NCONF = 128
NCORES = 8
BC = B // NCORES
RCLAMP = 1e6
CG = 8       # configs per gather chunk
CL = 32      # configs per LU chunk
# elimination steps whose S-update runs on GPSIMD, per stream (A, B).
# Empty: gpsimd TT thrashes the ext-isa IRAM against ap_gather (~6us
# reload per alternation) and stalls the DVE dependency chain.
KS_GPS_A = ()
KS_GPS_B = ()


def wrap_idx(idx: np.ndarray) -> np.ndarray:
    """Wrap a flat index list into ap_gather's [128, n/16] layout."""
    n = idx.shape[0]
    assert n % 16 == 0
    w = idx.reshape(n // 16, 16).T.astype(np.int16)
    return np.tile(w, (8, 1))


def build_gidx1(cfg: np.ndarray) -> np.ndarray:
    """Stage-1 indices per c8 chunk: (c,i,h) -> block cfg[c,i]*2+h."""
    cols = []
    for ch in range(NCONF // CG):
        sl = cfg[ch * CG:(ch + 1) * CG]                    # [CG, 16]
        idx = (sl[:, :, None].astype(np.int64) * 2
               + np.arange(2)[None, None, :]).reshape(-1)  # CG*K*2 = 256
        cols.append(wrap_idx(idx))                         # [128, 16]
    return np.concatenate(cols, axis=1)                    # [128, 256]


def build_gidx2(cfg: np.ndarray) -> np.ndarray:
    """Stage-2 indices per c8 chunk: (c_local, j) -> c_local*32 + cfg[c,j]."""
    cols = []
    for ch in range(NCONF // CG):
        sl = cfg[ch * CG:(ch + 1) * CG]                    # [CG, 16]
        idx = (np.arange(CG)[:, None] * NE + sl).reshape(-1)   # CG*K = 128
        cols.append(wrap_idx(idx))                         # [128, 8]
    return np.concatenate(cols, axis=1)                    # [128, 128]


def emit_program(nc, tc, aps, BCc: int):
    ctx = ExitStack()
    NBT = BCc // 128
    NCH = NCONF // CL          # LU chunks per btile (4)
    NG = CL // CG              # gather chunks per LU chunk (4)
    ao, w32t, ident, cirep, gidx1, gidx2, mscr, out = (
        aps["ao"], aps["w32t"], aps["ident"], aps["cirep"], aps["gidx1"],
        aps["gidx2"], aps["mscr"], aps["out"])

    with ctx:
        cpool = ctx.enter_context(tc.tile_pool(name="consts", bufs=1))
        nat = ctx.enter_context(tc.tile_pool(name="nat", bufs=4))
        tp_ps = ctx.enter_context(
            tc.tile_pool(name="tp_ps", bufs=3, space="PSUM"))
        aot = ctx.enter_context(tc.tile_pool(name="aot", bufs=2))
        m_ps = ctx.enter_context(
            tc.tile_pool(name="m_ps", bufs=3, space="PSUM"))
        msb = ctx.enter_context(tc.tile_pool(name="msb", bufs=1))
        rp = ctx.enter_context(tc.tile_pool(name="rp", bufs=2))
        rtp = ctx.enter_context(tc.tile_pool(name="rtp", bufs=1))
        subp = ctx.enter_context(tc.tile_pool(name="subp", bufs=1))
        lb = ctx.enter_context(tc.tile_pool(name="lb", bufs=1))
        pb = ctx.enter_context(tc.tile_pool(name="pb", bufs=1))
        sm = ctx.enter_context(tc.tile_pool(name="sm", bufs=2))
        dets = ctx.enter_context(tc.tile_pool(name="dets", bufs=2))
        outp = ctx.enter_context(tc.tile_pool(name="outp", bufs=1))

        w32t_s = cpool.tile([128, NE], F32)
        ident_s = cpool.tile([128, 128], F32)
        cirep_s = cpool.tile([128, NCONF], F32)
        gidx1_s = cpool.tile([128, NCONF * 2], I16)
        gidx2_s = cpool.tile([128, NCONF], I16)
        nc.sync.dma_start(w32t_s[:], w32t[:])
        nc.sync.dma_start(ident_s[:], ident[:])
        nc.sync.dma_start(cirep_s[:], cirep[:])
        nc.sync.dma_start(gidx1_s[:], gidx1[:])
        nc.sync.dma_start(gidx2_s[:], gidx2[:])

        out_sb = outp.tile([128, NBT], F32)

        ao3 = ao.rearrange("(t p) n -> t p n", p=128)
        # mscr: [32m, BC*32(b,e)] fp16 -- M^T layout
        mscr_r = mscr.rearrange("m (b e) -> b m e", e=NE)   # [BC, 32, 32]

        def lu_chunk_ops(sub_t, P_t, L_t, rec_t, ks_gps):
            """Per-step op emitter for one 32-cfg chunk.

            sub layout per partition: [CL, 16j, 16i] fp32 (transposed
            submatrix; det unchanged).  In-place elimination on (j, i)."""
            S4 = sub_t[:].rearrange("p (c j i) -> p c j i", j=K, i=K)
            L3 = L_t[:].rearrange("p (c i) -> p c i", c=CL)
            P4 = P_t[:].rearrange("p (c j i) -> p c j i", j=K - 1, i=K - 1)

            def step(k):
                r = K - 1 - k
                piv = S4[:, :, k, k]
                nc.vector.reciprocal(rec_t[:], piv)
                nc.vector.tensor_scalar(
                    rec_t[:], rec_t[:], -RCLAMP, RCLAMP,
                    op0=OP.max, op1=OP.min)
                # L[c,i] = col * rec  (col = S4[:, :, k, k+1:], i-dim)
                col = S4[:, :, k, k + 1:]
                Lv = L3[:, :, :r]
                nc.vector.tensor_tensor(
                    Lv, col,
                    rec_t[:].unsqueeze(2).broadcast_to([128, CL, r]),
                    op=OP.mult)
                # P[c,j,i] = row[c,j] x L[c,i]  (row = S4[:, :, k+1:, k])
                row = S4[:, :, k + 1:, k]
                Pv = P4[:, :, :r, :r]
                nc.vector.tensor_tensor(
                    Pv,
                    row.unsqueeze(3).broadcast_to([128, CL, r, r]),
                    Lv.unsqueeze(2).broadcast_to([128, CL, r, r]),
                    op=OP.mult)
                # S -= P
                Sv = S4[:, :, k + 1:, k + 1:]
                eng = nc.gpsimd if k in ks_gps else nc.vector
                eng.tensor_tensor(Sv, Sv, Pv, op=OP.subtract)
            return step

        def lu_finish(sub_t, dets_t, ch):
            """Trailing 2x2 det + product tree into dets_t[:, ch*CL:...]."""
            S4 = sub_t[:].rearrange("p (c j i) -> p c j i", j=K, i=K)
            t8 = sm.tile([128, CL * 8], F32, tag=f"t8{ch % 2}")
            t8v = t8[:].rearrange("p (c x) -> p c x", c=CL)
            # 7 diag pairs (k=0..13)
            d = sub_t[:]
            nc.vector.tensor_tensor(
                t8v[:, :, :7],
                bass.AP(d.tensor, d.offset,
                        [[int(d.ap[0][0]), 128], [K * K, CL], [34, 7]]),
                bass.AP(d.tensor, d.offset + 17,
                        [[int(d.ap[0][0]), 128], [K * K, CL], [34, 7]]),
                op=OP.mult)
            # det2 of trailing 2x2: S[14,14]*S[15,15] - S[14,15]*S[15,14]
            m1 = sm.tile([128, CL], F32, tag=f"m1{ch % 2}")
            nc.vector.tensor_tensor(
                m1[:], S4[:, :, K - 2, K - 2], S4[:, :, K - 1, K - 1],
                op=OP.mult)
            m2 = sm.tile([128, CL], F32, tag=f"m2{ch % 2}")
            nc.vector.tensor_tensor(
                m2[:], S4[:, :, K - 2, K - 1], S4[:, :, K - 1, K - 2],
                op=OP.mult)
            nc.vector.tensor_tensor(t8v[:, :, 7], m1[:], m2[:],
                                    op=OP.subtract)
            # tree 8 -> 4 -> 2 -> 1
            t4 = sm.tile([128, CL * 4], F32, tag=f"t4{ch % 2}")
            nc.vector.tensor_tensor(
                t4[:].rearrange("p (c x) -> p c x", c=CL),
                bass.AP(t8[:].tensor, t8[:].offset,
                        [[int(t8[:].ap[0][0]), 128], [8, CL], [2, 4]]),
                bass.AP(t8[:].tensor, t8[:].offset + 1,
                        [[int(t8[:].ap[0][0]), 128], [8, CL], [2, 4]]),
                op=OP.mult)
            t2 = sm.tile([128, CL * 2], F32, tag=f"t2{ch % 2}")
            nc.vector.tensor_tensor(
                t2[:].rearrange("p (c x) -> p c x", c=CL),
                bass.AP(t4[:].tensor, t4[:].offset,
                        [[int(t4[:].ap[0][0]), 128], [4, CL], [2, 2]]),
                bass.AP(t4[:].tensor, t4[:].offset + 1,
                        [[int(t4[:].ap[0][0]), 128], [4, CL], [2, 2]]),
                op=OP.mult)
            nc.vector.tensor_tensor(
                dets_t[:, ch * CL:(ch + 1) * CL],
                bass.AP(t2[:].tensor, t2[:].offset,
                        [[int(t2[:].ap[0][0]), 128], [2, CL]]),
                bass.AP(t2[:].tensor, t2[:].offset + 1,
                        [[int(t2[:].ap[0][0]), 128], [2, CL]]),
                op=OP.mult)

        def phase1(bt):
            """M^T = W32 @ ao^T, written to mscr in fp16."""
            for t in range(32):
                nat_t = nat.tile([128, 128], F32)
                nc.sync.dma_start(nat_t[:], ao3[bt * 32 + t])
                ps = tp_ps.tile([128, 128], F32)
                nc.tensor.transpose(ps[:], nat_t[:], ident_s[:])
                aot_t = aot.tile([128, 128], F32)
                nc.scalar.copy(aot_t[:], ps[:])
                mp = m_ps.tile([NE, 128], F32)
                nc.tensor.matmul(
                    mp[:], w32t_s[:], aot_t[:], start=True, stop=True)
                msb_s = nat.tile([NE, 128], F16, tag="mstage")
                nc.scalar.copy(msb_s[:], mp[:])
                nc.scalar.dma_start(
                    mscr[:, (bt * 128 + t * 4) * NE:
                         (bt * 128 + t * 4 + 4) * NE],
                    msb_s[:])

        msb_tiles = {}

        def get_msb(bt):
            if bt not in msb_tiles:
                t = msb.tile([128, NE * NE], F16, tag=f"m{bt % 2}")
                nc.sync.dma_start(t[:], mscr_r[bt * 128:(bt + 1) * 128])
                msb_tiles[bt] = t
            return msb_tiles[bt]

        sub_tiles = {}

        def gather_piece(bt, ch, g):
            """Emit one c8-chunk gather (g1 -> transpose -> g2)."""
            key = (bt, ch)
            if key not in sub_tiles:
                ci = bt * NCH + ch
                sub_tiles[key] = subp.tile(
                    [128, CL * K * K], F32, name=f"sub{ci % 3}",
                    tag=f"s{ci % 3}")
            sub_t = sub_tiles[key]
            msb_t = get_msb(bt)
            cg = ch * NG + g               # c8-chunk id within btile
            r_t = rp.tile([128, CG * K * NE], F16)
            nc.gpsimd.ap_gather(
                r_t[:], msb_t[:],
                gidx1_s[:, cg * 16:(cg + 1) * 16],
                channels=128, num_elems=NE * 2, d=16,
                num_idxs=CG * K * 2)
            # transpose + cast fp16 -> fp32: [c,i,m] -> [c,m,i]
            # walk order (c,i,m): contiguous reads, strided writes
            rt_t = rtp.tile([128, CG * K * NE], F32)
            rt_dst = bass.AP(
                rt_t[:].tensor, rt_t[:].offset,
                [[int(rt_t[:].ap[0][0]), 128],
                 [K * NE, CG], [1, K], [K, NE]])
            r_src = bass.AP(
                r_t[:].tensor, r_t[:].offset,
                [[int(r_t[:].ap[0][0]), 128],
                 [K * NE, CG], [NE, K], [1, NE]])
            nc.scalar.copy(rt_dst, r_src)
            nc.gpsimd.ap_gather(
                sub_t[:, g * CG * K * K:(g + 1) * CG * K * K],
                rt_t[:],
                gidx2_s[:, cg * 8:(cg + 1) * 8],
                channels=128, num_elems=CG * NE, d=16,
                num_idxs=CG * K)

        # flat chunk order over the whole program; gathers for chunk
        # pair p+1 are emitted inside pair p's k-loop (slots k=6..13)
        all_chunks = [(bt, ch) for bt in range(NBT) for ch in range(NCH)]
        pieces = [(bt, ch, g) for (bt, ch) in all_chunks for g in range(NG)]
        pos = 0                            # next piece to emit

        phase1(0)
        # prime the pipeline: chunks 0 and 1 of btile 0
        while pos < 2 * NG:
            bt_, ch_, g_ = pieces[pos]
            gather_piece(bt_, ch_, g_)
            pos += 1

        for bt in range(NBT):
            if bt + 1 < NBT:
                phase1(bt + 1)
            dets_t = dets.tile([128, NCONF], F32)
            for chp in range(NCH // 2):
                chA, chB = chp * 2, chp * 2 + 1
                subA = sub_tiles[(bt, chA)]
                subB = sub_tiles[(bt, chB)]
                # P/L/rec shared between streams: all LU ops run in-order
                # on DVE, so stream B's writes always follow stream A's
                # last read -- sharing adds no stalls and saves SBUF.
                P_t = pb.tile([128, CL * (K - 1) * (K - 1)], F32, tag="p")
                L_t = lb.tile([128, CL * (K - 1)], F32, tag="l")
                rec_t = sm.tile([128, CL], F32, tag="rec")
                stepA = lu_chunk_ops(subA, P_t, L_t, rec_t, KS_GPS_A)
                stepB = lu_chunk_ops(subB, P_t, L_t, rec_t, KS_GPS_B)
                npc = min(2 * NG, len(pieces) - pos)   # pieces this pair
                for k in range(K - 2):
                    stepA(k)
                    stepB(k)
                    # emit next-pair gathers early in the k-loop so the
                    # gather pipeline hides fully under this pair's LU
                    if k < npc:
                        bt_, ch_, g_ = pieces[pos]
                        gather_piece(bt_, ch_, g_)
                        pos += 1
                lu_finish(subA, dets_t, chA)
                lu_finish(subB, dets_t, chB)
                del sub_tiles[(bt, chA)]
                del sub_tiles[(bt, chB)]

            wd = sm.tile([128, NCONF], F32, tag="wd")
            nc.vector.tensor_tensor(wd[:], dets_t[:], cirep_s[:], op=OP.mult)
            nc.vector.tensor_reduce(
                out_sb[:, bt:bt + 1], wd[:], axis=AX.X, op=OP.add)
            msb_tiles.pop(bt, None)

        nc.sync.dma_start(out[:], out_sb[:])


def build(BCc: int):
    nc = bacc.Bacc("TRN2", target_bir_lowering=False, debug=False)
    aps = {}
    aps["ao"] = nc.dram_tensor(
        "ao", [BCc * NE, NAO], F32, kind="ExternalInput").ap()
    aps["w32t"] = nc.dram_tensor(
        "w32t", [NAO, NE], F32, kind="ExternalInput").ap()
    aps["ident"] = nc.dram_tensor(
        "ident", [128, 128], F32, kind="ExternalInput").ap()
    aps["cirep"] = nc.dram_tensor(
        "cirep", [128, NCONF], F32, kind="ExternalInput").ap()
    aps["gidx1"] = nc.dram_tensor(
        "gidx1", [128, NCONF * 2], I16, kind="ExternalInput").ap()
    aps["gidx2"] = nc.dram_tensor(
        "gidx2", [128, NCONF], I16, kind="ExternalInput").ap()
    aps["mscr"] = nc.dram_tensor("mscr", [NE, BCc * NE], F16).ap()
    aps["out"] = nc.dram_tensor(
        "out", [128, BCc // 128], F32, kind="ExternalOutput").ap()

    with tile.TileContext(nc) as tc:
        emit_program(nc, tc, aps, BCc)
    nc.compile()
    return nc


def host_inputs(ao_shard, mo_weight, ci_weight, configs):
    BCc = ao_shard.shape[0]
    w32 = mo_weight[:NE, :]
    return {
        "ao": np.ascontiguousarray(
            ao_shard.reshape(BCc * NE, NAO)).astype(np.float32),
        "w32t": np.ascontiguousarray(w32.T).astype(np.float32),
        "ident": np.eye(128, dtype=np.float32),
        "cirep": np.ascontiguousarray(
            np.tile(ci_weight.astype(np.float32), (128, 1))),
        "gidx1": build_gidx1(configs),
        "gidx2": build_gidx2(configs),
    }


_CACHE: dict = {}


def _get_program():
    key = ("prog", BC, CL, KS_GPS_A, KS_GPS_B)
    if key not in _CACHE:
        _CACHE[key] = build(BC)
    return _CACHE[key]


def kernel(ao, mo_weight, ci_weight, configs):
    ao = np.asarray(ao, dtype=np.float32)
    mo_weight = np.asarray(mo_weight, dtype=np.float32)
    ci_weight = np.asarray(ci_weight, dtype=np.float32)
    configs = np.asarray(configs, dtype=np.int32)
    assert ao.shape == (B, NE, NAO)

    nc = _get_program()
    in_maps = [
        host_inputs(ao[c * BC:(c + 1) * BC], mo_weight, ci_weight, configs)
        for c in range(NCORES)
    ]
    res = run_bass_kernel_spmd(nc, in_maps, core_ids=list(range(NCORES)))
    outs = []
    for c in range(NCORES):
        o = np.asarray(res.results[c]["out"])      # [128, NBT]
        outs.append(o.T.reshape(-1))               # b = bt*128 + p
    return np.concatenate(outs).astype(np.float32)[:, None]


def ref_algo(ao_shard, mo_weight, ci_weight, configs):
    """Numpy replica of the on-device algorithm (dev checking only)."""
    M = np.einsum("ben,mn->bem", ao_shard, mo_weight[:NE]).astype(np.float32)
    M = M.astype(np.float16).astype(np.float32)
    sub = M[:, configs[:, :, None], configs[:, None, :]].astype(np.float32)
    subT = np.swapaxes(sub, -1, -2)
    Bs = subT.shape[0]
    A = subT.reshape(-1, K, K).copy()
    rcl = np.float32(RCLAMP)
    dets = np.ones(A.shape[0], np.float32)
    for k in range(K - 2):
        piv = A[:, k, k].copy()
        with np.errstate(divide="ignore"):
            rec = (np.float32(1.0) / piv).astype(np.float32)
        rec = np.clip(rec, -rcl, rcl)
        L = (A[:, k, k + 1:] * rec[:, None]).astype(np.float32)
        A[:, k + 1:, k + 1:] -= (
            A[:, k + 1:, k][:, :, None] * L[:, None, :]).astype(np.float32)
        dets = (dets * piv).astype(np.float32)
    det2 = (A[:, K - 2, K - 2] * A[:, K - 1, K - 1]
            - A[:, K - 2, K - 1] * A[:, K - 1, K - 2]).astype(np.float32)
    dets = (dets * det2).astype(np.float32)
    dets_ = dets.reshape(Bs, NCONF)
    return (dets_ @ ci_weight.T.astype(np.float32)).astype(np.float32)


# revision 18
# speedup vs baseline: 1.3938x; 1.0274x over previous
"""Trainium2 Bass kernel for nn_NEURAL_PYSCF_WF (neural wavefunction).

reference:
  mo   = einsum('ben,mn->bem', ao, mo_weight)          # [B, 32, 128]
  sub  = mo[:, cfg[:,:,None], cfg[:,None,:]]           # [B, 128, 16, 16]
  dets = det(sub)                                      # [B, 128]
  out  = dets @ ci_weight.T                            # [B, 1]

Config indices are < 32, so only mo[:, :, :32] matters.

Strategy (8 NeuronCores, data-parallel over B=8192). Per core (1024 rows):
  phase 1: ao tiles -> PE transpose -> matmul (W32T stationary) ->
           M^T [m,(b,e)] in fp16 -> DRAM scratch; reload per 128-row
           b-tile as M [128b, 1024(e,m)] fp16.
  phase 2 per b-tile (128 walkers in partitions):
    per c8 sub-chunk: gpsimd ap_gather (d=16, fp16) config rows ->
      R [c,i,m32] fp16; ScalarE strided copy-transpose+cast ->
      Rt [c,m32,i16] fp32; gpsimd ap_gather (d=16, fp32) config cols ->
      sub [c,j,i] fp32 (det(A^T)==det(A)).
    LU: pivot-free elimination over chunks of 32 configs, two chunks
      interleaved so cross-engine gaps are filled.  Reciprocal clamped
      to +-1e6.  The S -= L*row update for the biggest steps
      (k in KS_GPS) runs on GPSIMD in parallel with DVE.
      Trailing 2x2 determinant in closed form; product tree in fp32.
  out[b] = sum_c ci[c] * det[b, c]  (TT mult + reduce).
"""

from contextlib import ExitStack

import numpy as np

import concourse.bass as bass
import concourse.bacc as bacc
import concourse.mybir as mybir
import concourse.tile as tile
from concourse.bass_utils import run_bass_kernel_spmd

F32 = mybir.dt.float32
F16 = mybir.dt.float16
I16 = mybir.dt.int16
AX = mybir.AxisListType
OP = mybir.AluOpType

B = 8192
NE = 32      # electrons (and the max config index)
NAO = 128
K = 16       # config size
NCONF = 128
NCORES = 8
BC = B // NCORES
RCLAMP = 1e6
CG = 8       # configs per gather chunk
CL = 32      # configs per LU chunk
# elimination steps whose S-update runs on GPSIMD, per stream (A, B).
# Empty: gpsimd TT thrashes the ext-isa IRAM against ap_gather (~6us
# reload per alternation) and stalls the DVE dependency chain.
KS_GPS_A = ()
KS_GPS_B = ()


def wrap_idx(idx: np.ndarray) -> np.ndarray:
    """Wrap a flat index list into ap_gather's [128, n/16] layout."""
    n = idx.shape[0]
    assert n % 16 == 0
    w = idx.reshape(n // 16, 16).T.astype(np.int16)
    return np.tile(w, (8, 1))


def build_gidx1(cfg: np.ndarray) -> np.ndarray:
    """Stage-1 indices per c8 chunk: (c,i,h) -> block cfg[c,i]*2+h."""
    cols = []
    for ch in range(NCONF // CG):
        sl = cfg[ch * CG:(ch + 1) * CG]                    # [CG, 16]
        idx = (sl[:, :, None].astype(np.int64) * 2
               + np.arange(2)[None, None, :]).reshape(-1)  # CG*K*2 = 256
        cols.append(wrap_idx(idx))                         # [128, 16]
    return np.concatenate(cols, axis=1)                    # [128, 256]


def build_gidx2(cfg: np.ndarray) -> np.ndarray:
    """Stage-2 indices per c8 chunk: (c_local, j) -> c_local*32 + cfg[c,j]."""
    cols = []
    for ch in range(NCONF // CG):
        sl = cfg[ch * CG:(ch + 1) * CG]                    # [CG, 16]
        idx = (np.arange(CG)[:, None] * NE + sl).reshape(-1)   # CG*K = 128
        cols.append(wrap_idx(idx))                         # [128, 8]
    return np.concatenate(cols, axis=1)                    # [128, 128]


def emit_program(nc, tc, aps, BCc: int):
    ctx = ExitStack()
    NBT = BCc // 128
    NCH = NCONF // CL          # LU chunks per btile (4)
    NG = CL // CG              # gather chunks per LU chunk (4)
    ao, w32t, ident, cirep, gidx1, gidx2, mscr, out = (
        aps["ao"], aps["w32t"], aps["ident"], aps["cirep"], aps["gidx1"],
        aps["gidx2"], aps["mscr"], aps["out"])

    with ctx:
        cpool = ctx.enter_context(tc.tile_pool(name="consts", bufs=1))
        nat = ctx.enter_context(tc.tile_pool(name="nat", bufs=4))
        tp_ps = ctx.enter_context(
            tc.tile_pool(name="tp_ps", bufs=3, space="PSUM"))
        aot = ctx.enter_context(tc.tile_pool(name="aot", bufs=2))
        m_ps = ctx.enter_context(
            tc.tile_pool(name="m_ps", bufs=3, space="PSUM"))
        msb = ctx.enter_context(tc.tile_pool(name="msb", bufs=1))
        rp = ctx.enter_context(tc.tile_pool(name="rp", bufs=2))
        rtp = ctx.enter_context(tc.tile_pool(name="rtp", bufs=1))
        subp = ctx.enter_context(tc.tile_pool(name="subp", bufs=1))
        lb = ctx.enter_context(tc.tile_pool(name="lb", bufs=1))
        pb = ctx.enter_context(tc.tile_pool(name="pb", bufs=1))
        sm = ctx.enter_context(tc.tile_pool(name="sm", bufs=1))
        dets = ctx.enter_context(tc.tile_pool(name="dets", bufs=2))
        outp = ctx.enter_context(tc.tile_pool(name="outp", bufs=1))

        w32t_s = cpool.tile([128, NE], F32)
        ident_s = cpool.tile([128, 128], F32)
        cirep_s = cpool.tile([128, NCONF], F32)
        gidx1_s = cpool.tile([128, NCONF * 2], I16)
        gidx2_s = cpool.tile([128, NCONF], I16)
        nc.sync.dma_start(w32t_s[:], w32t[:])
        nc.sync.dma_start(ident_s[:], ident[:])
        nc.sync.dma_start(cirep_s[:], cirep[:])
        nc.sync.dma_start(gidx1_s[:], gidx1[:])
        nc.sync.dma_start(gidx2_s[:], gidx2[:])

        out_sb = outp.tile([128, NBT], F32)

        ao3 = ao.rearrange("(t p) n -> t p n", p=128)
        # mscr: [32m, BC*32(b,e)] fp16 -- M^T layout
        mscr_r = mscr.rearrange("m (b e) -> b m e", e=NE)   # [BC, 32, 32]

        def lu_chunk_ops(sub_t, P_t, L_t, rec_t, ks_gps):
            """Per-step op emitter for one 32-cfg chunk.

            sub layout per partition: [CL, 16j, 16i] fp32 (transposed
            submatrix; det unchanged).  In-place elimination on (j, i)."""
            S4 = sub_t[:].rearrange("p (c j i) -> p c j i", j=K, i=K)
            L3 = L_t[:].rearrange("p (c i) -> p c i", c=CL)
            P4 = P_t[:].rearrange("p (c j i) -> p c j i", j=K - 1, i=K - 1)

            def step(k):
                r = K - 1 - k
                piv = S4[:, :, k, k]
                nc.vector.reciprocal(rec_t[:], piv)
                nc.vector.tensor_scalar(
                    rec_t[:], rec_t[:], -RCLAMP, RCLAMP,
                    op0=OP.max, op1=OP.min)
                # L[c,i] = col * rec  (col = S4[:, :, k, k+1:], i-dim)
                col = S4[:, :, k, k + 1:]
                Lv = L3[:, :, :r]
                nc.vector.tensor_tensor(
                    Lv, col,
                    rec_t[:].unsqueeze(2).broadcast_to([128, CL, r]),
                    op=OP.mult)
                # P[c,j,i] = row[c,j] x L[c,i]  (row = S4[:, :, k+1:, k])
                row = S4[:, :, k + 1:, k]
                Pv = P4[:, :, :r, :r]
                nc.vector.tensor_tensor(
                    Pv,
                    row.unsqueeze(3).broadcast_to([128, CL, r, r]),
                    Lv.unsqueeze(2).broadcast_to([128, CL, r, r]),
                    op=OP.mult)
                # S -= P
                Sv = S4[:, :, k + 1:, k + 1:]
                eng = nc.gpsimd if k in ks_gps else nc.vector
                eng.tensor_tensor(Sv, Sv, Pv, op=OP.subtract)
            return step

        def lu_finish(sub_t, dets_t, ch):
            """Trailing 2x2 det + product tree into dets_t[:, ch*CL:...]."""
            S4 = sub_t[:].rearrange("p (c j i) -> p c j i", j=K, i=K)
            t8 = sm.tile([128, CL * 8], F32, tag=f"t8{ch % 2}")
            t8v = t8[:].rearrange("p (c x) -> p c x", c=CL)
            # 7 diag pairs (k=0..13)
            d = sub_t[:]
            nc.vector.tensor_tensor(
                t8v[:, :, :7],
                bass.AP(d.tensor, d.offset,
                        [[int(d.ap[0][0]), 128], [K * K, CL], [34, 7]]),
                bass.AP(d.tensor, d.offset + 17,
                        [[int(d.ap[0][0]), 128], [K * K, CL], [34, 7]]),
                op=OP.mult)
            # det2 of trailing 2x2: S[14,14]*S[15,15] - S[14,15]*S[15,14]
            m1 = sm.tile([128, CL], F32, tag=f"m1{ch % 2}")
            nc.vector.tensor_tensor(
                m1[:], S4[:, :, K - 2, K - 2], S4[:, :, K - 1, K - 1],
                op=OP.mult)
            m2 = sm.tile([128, CL], F32, tag=f"m2{ch % 2}")
            nc.vector.tensor_tensor(
                m2[:], S4[:, :, K - 2, K - 1], S4[:, :, K - 1, K - 2],
                op=OP.mult)
            nc.vector.tensor_tensor(t8v[:, :, 7], m1[:], m2[:],
                                    op=OP.subtract)
            # tree 8 -> 4 -> 2 -> 1
            t4 = sm.tile([128, CL * 4], F32, tag=f"t4{ch % 2}")
            nc.vector.tensor_tensor(
                t4[:].rearrange("p (c x) -> p c x", c=CL),
                bass.AP(t8[:].tensor, t8[:].offset,
                        [[int(t8[:].ap[0][0]), 128], [8, CL], [2, 4]]),
                bass.AP(t8[:].tensor, t8[:].offset + 1,
                        [[int(t8[:].ap[0][0]), 128], [8, CL], [2, 4]]),
                op=OP.mult)
            t2 = sm.tile([128, CL * 2], F32, tag=f"t2{ch % 2}")
            nc.vector.tensor_tensor(
                t2[:].rearrange("p (c x) -> p c x", c=CL),
                bass.AP(t4[:].tensor, t4[:].offset,
                        [[int(t4[:].ap[0][0]), 128], [4, CL], [2, 2]]),
                bass.AP(t4[:].tensor, t4[:].offset + 1,
                        [[int(t4[:].ap[0][0]), 128], [4, CL], [2, 2]]),
                op=OP.mult)
            nc.vector.tensor_tensor(
                dets_t[:, ch * CL:(ch + 1) * CL],
                bass.AP(t2[:].tensor, t2[:].offset,
                        [[int(t2[:].ap[0][0]), 128], [2, CL]]),
                bass.AP(t2[:].tensor, t2[:].offset + 1,
                        [[int(t2[:].ap[0][0]), 128], [2, CL]]),
                op=OP.mult)

        def phase1(bt):
            """M^T = W32 @ ao^T, written to mscr in fp16."""
            for t in range(32):
                nat_t = nat.tile([128, 128], F32)
                nc.sync.dma_start(nat_t[:], ao3[bt * 32 + t])
                ps = tp_ps.tile([128, 128], F32)
                nc.tensor.transpose(ps[:], nat_t[:], ident_s[:])
                aot_t = aot.tile([128, 128], F32)
                nc.scalar.copy(aot_t[:], ps[:])
                mp = m_ps.tile([NE, 128], F32)
                nc.tensor.matmul(
                    mp[:], w32t_s[:], aot_t[:], start=True, stop=True)
                msb_s = nat.tile([NE, 128], F16, tag="mstage")
                nc.scalar.copy(msb_s[:], mp[:])
                nc.scalar.dma_start(
                    mscr[:, (bt * 128 + t * 4) * NE:
                         (bt * 128 + t * 4 + 4) * NE],
                    msb_s[:])

        msb_tiles = {}

        def get_msb(bt):
            if bt not in msb_tiles:
                t = msb.tile([128, NE * NE], F16, tag=f"m{bt % 2}")
                nc.sync.dma_start(t[:], mscr_r[bt * 128:(bt + 1) * 128])
                msb_tiles[bt] = t
            return msb_tiles[bt]

        sub_tiles = {}

        def gather_piece(bt, ch, g):
            """Emit one c8-chunk gather (g1 -> transpose -> g2)."""
            key = (bt, ch)
            if key not in sub_tiles:
                ci = bt * NCH + ch
                sub_tiles[key] = subp.tile(
                    [128, CL * K * K], F32, name=f"sub{ci % 4}",
                    tag=f"s{ci % 4}")
            sub_t = sub_tiles[key]
            msb_t = get_msb(bt)
            cg = ch * NG + g               # c8-chunk id within btile
            r_t = rp.tile([128, CG * K * NE], F16)
            nc.gpsimd.ap_gather(
                r_t[:], msb_t[:],
                gidx1_s[:, cg * 16:(cg + 1) * 16],
                channels=128, num_elems=NE * 2, d=16,
                num_idxs=CG * K * 2)
            # transpose + cast fp16 -> fp32: [c,i,m] -> [c,m,i]
            # walk order (c,i,m): contiguous reads, strided writes
            rt_t = rtp.tile([128, CG * K * NE], F32)
            rt_dst = bass.AP(
                rt_t[:].tensor, rt_t[:].offset,
                [[int(rt_t[:].ap[0][0]), 128],
                 [K * NE, CG], [1, K], [K, NE]])
            r_src = bass.AP(
                r_t[:].tensor, r_t[:].offset,
                [[int(r_t[:].ap[0][0]), 128],
                 [K * NE, CG], [NE, K], [1, NE]])
            nc.scalar.copy(rt_dst, r_src)
            nc.gpsimd.ap_gather(
                sub_t[:, g * CG * K * K:(g + 1) * CG * K * K],
                rt_t[:],
                gidx2_s[:, cg * 8:(cg + 1) * 8],
                channels=128, num_elems=CG * NE, d=16,
                num_idxs=CG * K)

        # flat chunk order over the whole program; gathers for chunk
        # pair p+1 are emitted inside pair p's k-loop (slots k=6..13)
        all_chunks = [(bt, ch) for bt in range(NBT) for ch in range(NCH)]
        pieces = [(bt, ch, g) for (bt, ch) in all_chunks for g in range(NG)]
        pos = 0                            # next piece to emit

        phase1(0)
        # prime the pipeline: chunks 0 and 1 of btile 0
        while pos < 2 * NG:
            bt_, ch_, g_ = pieces[pos]
            gather_piece(bt_, ch_, g_)
            pos += 1

        for bt in range(NBT):
            if bt + 1 < NBT:
                phase1(bt + 1)
            dets_t = dets.tile([128, NCONF], F32)
            for chp in range(NCH // 2):
                chA, chB = chp * 2, chp * 2 + 1
                subA = sub_tiles[(bt, chA)]
                subB = sub_tiles[(bt, chB)]
                # P/L/rec shared between streams: all LU ops run in-order
                # on DVE, so stream B's writes always follow stream A's
                # last read -- sharing adds no stalls and saves SBUF.
                P_t = pb.tile([128, CL * (K - 1) * (K - 1)], F32, tag="p")
                L_t = lb.tile([128, CL * (K - 1)], F32, tag="l")
                rec_t = sm.tile([128, CL], F32, tag="rec")
                stepA = lu_chunk_ops(subA, P_t, L_t, rec_t, KS_GPS_A)
                stepB = lu_chunk_ops(subB, P_t, L_t, rec_t, KS_GPS_B)
                npc = min(2 * NG, len(pieces) - pos)   # pieces this pair
                for k in range(K - 2):
                    stepA(k)
                    stepB(k)
                    # emit next-pair gathers early in the k-loop so the
                    # gather pipeline hides fully under this pair's LU
                    if k < npc:
                        bt_, ch_, g_ = pieces[pos]
                        gather_piece(bt_, ch_, g_)
                        pos += 1
                lu_finish(subA, dets_t, chA)
                lu_finish(subB, dets_t, chB)
                del sub_tiles[(bt, chA)]
                del sub_tiles[(bt, chB)]

            wd = sm.tile([128, NCONF], F32, tag="wd")
            nc.vector.tensor_tensor(wd[:], dets_t[:], cirep_s[:], op=OP.mult)
            nc.vector.tensor_reduce(
                out_sb[:, bt:bt + 1], wd[:], axis=AX.X, op=OP.add)
            msb_tiles.pop(bt, None)

        nc.sync.dma_start(out[:], out_sb[:])


def build(BCc: int):
    nc = bacc.Bacc("TRN2", target_bir_lowering=False, debug=False)
    aps = {}
    aps["ao"] = nc.dram_tensor(
        "ao", [BCc * NE, NAO], F32, kind="ExternalInput").ap()
    aps["w32t"] = nc.dram_tensor(
        "w32t", [NAO, NE], F32, kind="ExternalInput").ap()
    aps["ident"] = nc.dram_tensor(
        "ident", [128, 128], F32, kind="ExternalInput").ap()
    aps["cirep"] = nc.dram_tensor(
        "cirep", [128, NCONF], F32, kind="ExternalInput").ap()
    aps["gidx1"] = nc.dram_tensor(
        "gidx1", [128, NCONF * 2], I16, kind="ExternalInput").ap()
    aps["gidx2"] = nc.dram_tensor(
        "gidx2", [128, NCONF], I16, kind="ExternalInput").ap()
    aps["mscr"] = nc.dram_tensor("mscr", [NE, BCc * NE], F16).ap()
    aps["out"] = nc.dram_tensor(
        "out", [128, BCc // 128], F32, kind="ExternalOutput").ap()

    with tile.TileContext(nc) as tc:
        emit_program(nc, tc, aps, BCc)
    nc.compile()
    return nc


def host_inputs(ao_shard, mo_weight, ci_weight, configs):
    BCc = ao_shard.shape[0]
    w32 = mo_weight[:NE, :]
    return {
        "ao": np.ascontiguousarray(
            ao_shard.reshape(BCc * NE, NAO)).astype(np.float32),
        "w32t": np.ascontiguousarray(w32.T).astype(np.float32),
        "ident": np.eye(128, dtype=np.float32),
        "cirep": np.ascontiguousarray(
            np.tile(ci_weight.astype(np.float32), (128, 1))),
        "gidx1": build_gidx1(configs),
        "gidx2": build_gidx2(configs),
    }


_CACHE: dict = {}


def _get_program():
    key = ("prog", BC, CL, KS_GPS_A, KS_GPS_B)
    if key not in _CACHE:
        _CACHE[key] = build(BC)
    return _CACHE[key]


def kernel(ao, mo_weight, ci_weight, configs):
    ao = np.asarray(ao, dtype=np.float32)
    mo_weight = np.asarray(mo_weight, dtype=np.float32)
    ci_weight = np.asarray(ci_weight, dtype=np.float32)
    configs = np.asarray(configs, dtype=np.int32)
    assert ao.shape == (B, NE, NAO)

    nc = _get_program()
    in_maps = [
        host_inputs(ao[c * BC:(c + 1) * BC], mo_weight, ci_weight, configs)
        for c in range(NCORES)
    ]
    res = run_bass_kernel_spmd(nc, in_maps, core_ids=list(range(NCORES)))
    outs = []
    for c in range(NCORES):
        o = np.asarray(res.results[c]["out"])      # [128, NBT]
        outs.append(o.T.reshape(-1))               # b = bt*128 + p
    return np.concatenate(outs).astype(np.float32)[:, None]


def ref_algo(ao_shard, mo_weight, ci_weight, configs):
    """Numpy replica of the on-device algorithm (dev checking only)."""
    M = np.einsum("ben,mn->bem", ao_shard, mo_weight[:NE]).astype(np.float32)
    M = M.astype(np.float16).astype(np.float32)
    sub = M[:, configs[:, :, None], configs[:, None, :]].astype(np.float32)
    subT = np.swapaxes(sub, -1, -2)
    Bs = subT.shape[0]
    A = subT.reshape(-1, K, K).copy()
    rcl = np.float32(RCLAMP)
    dets = np.ones(A.shape[0], np.float32)
    for k in range(K - 2):
        piv = A[:, k, k].copy()
        with np.errstate(divide="ignore"):
            rec = (np.float32(1.0) / piv).astype(np.float32)
        rec = np.clip(rec, -rcl, rcl)
        L = (A[:, k, k + 1:] * rec[:, None]).astype(np.float32)
        A[:, k + 1:, k + 1:] -= (
            A[:, k + 1:, k][:, :, None] * L[:, None, :]).astype(np.float32)
        dets = (dets * piv).astype(np.float32)
    det2 = (A[:, K - 2, K - 2] * A[:, K - 1, K - 1]
            - A[:, K - 2, K - 1] * A[:, K - 1, K - 2]).astype(np.float32)
    dets = (dets * det2).astype(np.float32)
    dets_ = dets.reshape(Bs, NCONF)
    return (dets_ @ ci_weight.T.astype(np.float32)).astype(np.float32)
